# revision 1
# baseline (speedup 1.0000x reference)
import sys
sys.path.insert(0, '/opt/trn_rl_repo')
import numpy as np
import math

import concourse.bass as bass
import concourse.mybir as mybir
import concourse.tile as tile
from concourse import bacc
from concourse.bass_utils import run_bass_kernel_spmd

import ml_dtypes
BF16NP = ml_dtypes.bfloat16

# Problem dims
B, SL, CH, HZ = 128, 5000, 12, 100
L, D, DFF, H, NCLS = 5, 1024, 4096, 16, 71
NI = CH * HZ          # 1200
S = SL // HZ          # 50
NCORES = 8
NB = B // NCORES      # 16 batches per core
T = NB * S            # 800 tokens per core
NIP = 1280            # padded input-feature dim
NKI = NIP // 128      # 10 input k-chunks
DK = D // H           # 64
NDC = D // 128        # 8 d-chunks
NFC = DFF // 128      # 32 ff-chunks
HB = NB // 2          # 8 batches per half
HT = HB * S           # 400 tokens per half

F32R = mybir.dt.float32r
F32 = mybir.dt.float32
BF16 = mybir.dt.bfloat16
EXP = mybir.ActivationFunctionType.Exp
RELU = mybir.ActivationFunctionType.Relu
COPY = mybir.ActivationFunctionType.Copy
AOP = mybir.AluOpType

TRACE = False
LAST_EXEC_NS = None
_CACHE = {}
FP8_FFN = False
TRIM_L5 = False
FP8 = mybir.dt.float8e4
FP8NP = ml_dtypes.float8_e4m3
DR = mybir.MatmulPerfMode.DoubleRow


def _build(n_layers=L):
    nc = bacc.Bacc(None)
    d = {}
    d['xT'] = nc.dram_tensor("xT", [NKI, 128, T], BF16, kind="ExternalInput")
    d['ones'] = nc.dram_tensor("ones", [128, 512], F32R, kind="ExternalInput")
    d['onesb'] = nc.dram_tensor("onesb", [128, 512], BF16, kind="ExternalInput")
    d['ident'] = nc.dram_tensor("ident", [128, 128], BF16, kind="ExternalInput")
    d['emb_w'] = nc.dram_tensor("emb_w", [NDC, 128, NKI * 128], BF16, kind="ExternalInput")
    d['qkv_w'] = nc.dram_tensor("qkv_w", [L, 3, NDC, 128, NDC * 128], BF16, kind="ExternalInput")
    d['qkv_bT'] = nc.dram_tensor("qkv_bT", [L, 128, 3 * NDC], F32, kind="ExternalInput")
    d['wo_w'] = nc.dram_tensor("wo_w", [L, NDC, 128, NDC * 128], BF16, kind="ExternalInput")
    d['wo_b'] = nc.dram_tensor("wo_b", [L, NDC, 1, 128], BF16, kind="ExternalInput")
    d['w1_bT'] = nc.dram_tensor("w1_bT", [L, 128, NFC], F32, kind="ExternalInput")
    d['w2_b'] = nc.dram_tensor("w2_b", [L, NDC, 1, 128], BF16, kind="ExternalInput")
    if FP8_FFN:
        d['w1_w8'] = nc.dram_tensor("w1_w8", [L, NFC, 128, NDC * 128], FP8, kind="ExternalInput")
        d['w2_w8'] = nc.dram_tensor("w2_w8", [L, NDC, 128, NFC * 128], FP8, kind="ExternalInput")
    else:
        d['w1_w'] = nc.dram_tensor("w1_w", [L, NFC, 128, NDC * 128], BF16, kind="ExternalInput")
        d['w2_w'] = nc.dram_tensor("w2_w", [L, NDC, 128, NFC * 128], BF16, kind="ExternalInput")
    d['cf_w'] = nc.dram_tensor("cf_w", [NDC, 128, NDC * 128], BF16, kind="ExternalInput")
    d['cf_bT'] = nc.dram_tensor("cf_bT", [128, NDC], F32, kind="ExternalInput")
    d['fc_w'] = nc.dram_tensor("fc_w", [128, NDC * NCLS], BF16, kind="ExternalInput")
    d['fc_b'] = nc.dram_tensor("fc_b", [NCLS, 1], F32, kind="ExternalInput")
    out = nc.dram_tensor("out", [NCLS, NB], F32, kind="ExternalOutput")

    with tile.TileContext(nc) as tc:
        _emit(nc, tc, d, out, n_layers)
    nc.compile()
    return nc


def _emit(nc, tc, d, out, n_layers):
    import contextlib
    ctx = contextlib.ExitStack()
    with ctx:
        sb1 = ctx.enter_context(tc.tile_pool(name="sb1", bufs=1))
        sq_p = ctx.enter_context(tc.tile_pool(name="sqp", bufs=2))
        wsm = ctx.enter_context(tc.tile_pool(name="wsm", bufs=8))
        wst = ctx.enter_context(tc.tile_pool(name="wst", bufs=3))
        rows = ctx.enter_context(tc.tile_pool(name="rows", bufs=4))
        rden_p = ctx.enter_context(tc.tile_pool(name="rden", bufs=2))
        brow_p = ctx.enter_context(tc.tile_pool(name="brow", bufs=2))
        pt_p = ctx.enter_context(tc.tile_pool(name="ptp", bufs=2))
        ptn_p = ctx.enter_context(tc.tile_pool(name="ptnp", bufs=2))
        ps_mm = ctx.enter_context(tc.tile_pool(name="psmm", bufs=2, space="PSUM"))
        ps_at = ctx.enter_context(tc.tile_pool(name="psat", bufs=4, space="PSUM"))
        ps_row = ctx.enter_context(tc.tile_pool(name="psrow", bufs=2, space="PSUM"))

        # persistent tiles
        hT = sb1.tile([128, NDC, T], F32R, tag="hT")
        ones_c = sb1.tile([128, 1], F32R, tag="ones_c")
        ones_r = sb1.tile([1, 512], F32R, tag="ones_r")
        onesb_c = sb1.tile([128, 1], BF16, tag="onesb_c")
        onesb_r = sb1.tile([1, 512], BF16, tag="onesb_r")
        ident = sb1.tile([128, 128], BF16, tag="ident")
        nc.sync.dma_start(ones_c[:], d['ones'][:, 0:1])
        nc.sync.dma_start(ones_r[:], d['ones'][0:1, :])
        nc.sync.dma_start(onesb_c[:], d['onesb'][:, 0:1])
        nc.sync.dma_start(onesb_r[:], d['onesb'][0:1, :])
        nc.sync.dma_start(ident[:], d['ident'][:, :])

        def ln_half(src, hcol0, ncols, dst, nch):
            """LN over feature dim (nch*128) of src[:, :, hcol0:hcol0+ncols] ->
            dst[:, :, 0:ncols] (bf16). src is [128, nch, *] fp32r."""
            Dn = float(nch * 128)
            cm = 1.0 / Dn
            cv2 = 1.0 / (Dn - 1.0)
            cv1 = -1.0 / (Dn * (Dn - 1.0))
            s1 = ps_row.tile([1, ncols], F32, tag="row")
            s2 = ps_row.tile([1, ncols], F32, tag="row")
            for c in range(nch):
                sq = sq_p.tile([128, ncols], F32R, tag="sq")
                nc.scalar.square(sq[:], src[:, c, hcol0:hcol0 + ncols])
                nc.tensor.matmul(s1[:], ones_c[:], src[:, c, hcol0:hcol0 + ncols],
                                 start=(c == 0), stop=(c == nch - 1))
                nc.tensor.matmul(s2[:], ones_c[:], sq[:],
                                 start=(c == 0), stop=(c == nch - 1))
            m_row = rows.tile([1, ncols], F32R, tag="rowsb")
            t1 = rows.tile([1, ncols], F32, tag="rowsb")
            t2 = rows.tile([1, ncols], F32, tag="rowsb")
            nc.vector.tensor_scalar_mul(m_row[:], s1[:], cm)
            nc.scalar.square(t1[:], s1[:])
            nc.vector.tensor_scalar_mul(t1[:], t1[:], cv1)
            nc.vector.tensor_scalar_mul(t2[:], s2[:], cv2)
            nc.vector.tensor_tensor(out=t1[:], in0=t1[:], in1=t2[:], op=AOP.add)
            nc.scalar.sqrt(t1[:], t1[:])
            nc.vector.tensor_scalar_add(t1[:], t1[:], 1e-6)
            r_row = rows.tile([1, ncols], F32R, tag="rowsb")
            with nc.allow_low_precision(reason="fp32r rounding of 1/(std+eps)"):
                nc.vector.reciprocal(r_row[:], t1[:])
            Mb = ps_at.tile([128, ncols], F32, tag="at")
            Rb = ps_at.tile([128, ncols], F32, tag="at")
            nc.tensor.matmul(Mb[:], ones_r[0:1, 0:128], m_row[:], start=True, stop=True)
            nc.tensor.matmul(Rb[:], ones_r[0:1, 0:128], r_row[:], start=True, stop=True)
            for c in range(nch):
                dslice = dst[:, c, hcol0:hcol0 + ncols]
                nc.vector.tensor_tensor(out=dslice, in0=src[:, c, hcol0:hcol0 + ncols],
                                        in1=Mb[:], op=AOP.subtract)
                nc.vector.tensor_tensor(out=dslice, in0=dslice,
                                        in1=Rb[:], op=AOP.mult)

        def ln_full(src, dst, nch=NDC, ncols_tot=T):
            for hf in range(ncols_tot // HT):
                ln_half(src, hf * HT, HT, dst, nch)

        # ---------------- embed ----------------
        xt = sb1.tile([128, NKI, T], BF16, tag="tagX")
        nc.sync.dma_start(xt[:], d['xT'].rearrange("k p t -> p k t"))
        for m in range(NDC):
            wt = wst.tile([128, NKI, 128], BF16, tag="wste")
            nc.sync.dma_start(wt[:], d['emb_w'][m].rearrange("p (k c) -> p k c", k=NKI))
            for hf in range(2):
                ps = ps_mm.tile([128, HT], F32, tag="mm")
                for k in range(NKI):
                    nc.tensor.matmul(ps[:], wt[:, k, :], xt[:, k, hf * HT:(hf + 1) * HT],
                                     start=(k == 0), stop=(k == NKI - 1))
                nc.vector.tensor_copy(hT[:, m, hf * HT:(hf + 1) * HT], ps[:])

        # ---------------- layers ----------------
        for li in range(n_layers):
            last = (li == n_layers - 1) and (n_layers == L)
            trim = last and TRIM_L5
            # ---- LN1 over full T ----
            aT = sb1.tile([128, NDC, T], BF16, tag="tagA")
            ln_full(hT, aT)
            # ---- Q, K (single weight pass, both halves) ----
            kT = sb1.tile([128, NDC, T], BF16, tag="tagK")
            bT = brow_p.tile([128, 2 * NDC], F32, tag="brow")
            nc.sync.dma_start(bT[:], d['qkv_bT'][li, :, 0:2 * NDC])
            if not trim:
                qT = sb1.tile([128, NDC, T], BF16, tag="tagQ")
                mats = ((0, qT), (1, kT))
            else:
                mats = ((1, kT),)
            for mat, dst in mats:
                for m in range(NDC):
                    wt = wsm.tile([128, NDC, 128], BF16, tag="wsm")
                    nc.sync.dma_start(wt[:], d['qkv_w'][li, mat, m].rearrange("p (k c) -> p k c", k=NDC))
                    for hf in range(2):
                        ps = ps_mm.tile([128, HT], F32, tag="mm")
                        for k in range(NDC):
                            nc.tensor.matmul(ps[:], wt[:, k, :], aT[:, k, hf * HT:(hf + 1) * HT],
                                             start=(k == 0), stop=(k == NDC - 1))
                        nc.vector.tensor_scalar_add(
                            dst[:, m, hf * HT:(hf + 1) * HT], ps[:],
                            bT[:, mat * NDC + m:mat * NDC + m + 1])
            if trim:
                # Q only for the last token of each batch
                aL16 = sb1.tile([128, NDC, NB], BF16, tag="aL16")
                nc.vector.tensor_copy(
                    aL16[:],
                    aT[:].rearrange("p c (b s) -> p c b s", s=S)[:, :, :, S - 1])
                qL = sb1.tile([128, NDC, NB], BF16, tag="tagQ")
                for m in range(NDC):
                    wt = wsm.tile([128, NDC, 128], BF16, tag="wsm")
                    nc.sync.dma_start(wt[:], d['qkv_w'][li, 0, m].rearrange("p (k c) -> p k c", k=NDC))
                    ps = ps_mm.tile([128, NB], F32, tag="mm")
                    for k in range(NDC):
                        nc.tensor.matmul(ps[:], wt[:, k, :], aL16[:, k, :],
                                         start=(k == 0), stop=(k == NDC - 1))
                    nc.vector.tensor_scalar_add(qL[:, m, :], ps[:], bT[:, m:m + 1])
            # ---- V: feature-major GEMM then per-batch transposes ----
            vTf = sb1.tile([128, NDC, T], BF16, tag="tagVf")
            for m in range(NDC):
                wt = wsm.tile([128, NDC, 128], BF16, tag="wsm")
                nc.sync.dma_start(wt[:], d['qkv_w'][li, 2, m].rearrange("p (k c) -> p k c", k=NDC))
                for hf in range(2):
                    ps = ps_mm.tile([128, HT], F32, tag="mm")
                    for k in range(NDC):
                        nc.tensor.matmul(ps[:], wt[:, k, :], aT[:, k, hf * HT:(hf + 1) * HT],
                                         start=(k == 0), stop=(k == NDC - 1))
                    nc.vector.tensor_copy(vTf[:, m, hf * HT:(hf + 1) * HT], ps[:])
            # v token-major, replicated: rows 0-49 and 64-113 (bf16)
            v = sb1.tile([128, NB, D], BF16, tag="tagV")
            for bi in range(NB):
                bc0 = bi * S
                pv = ps_at.tile([128, D], BF16, tag="at")
                for c in range(NDC):
                    nc.tensor.transpose(pv[0:S, c * 128:(c + 1) * 128],
                                        vTf[:, c, bc0:bc0 + S], ident[:],
                                        tile_position=(0, 0))
                nc.vector.tensor_copy(v[0:S, bi, :], pv[0:S, :])
                # odd-head replica rows 64-113 via partition-shift DMA
                nc.sync.dma_start(v[64:64 + S, bi, :], v[0:S, bi, :])
            if trim:
                # extract last-token residual before adding attention output
                hL = sb1.tile([128, NDC, NB], F32R, tag="hL")
                for c in range(NDC):
                    nc.vector.tensor_copy(
                        hL[:, c, :],
                        hT[:, c, :].rearrange("p (b s) -> p b s", s=S)[:, :, S - 1])
                # tiny attention: 1 query per batch
                oL = sb1.tile([128, NDC, NB], BF16, tag="tagO")
                for bi in range(NB):
                    bc0 = bi * S
                    psS = ps_at.tile([128, NDC], F32, tag="at")
                    for c in range(NDC):
                        nc.tensor.matmul(psS[0:S, c:c + 1],
                                         kT[0:DK, c, bc0:bc0 + S], qL[0:DK, c, bi:bi + 1],
                                         start=True, stop=True)
                    for c in range(NDC):
                        nc.tensor.matmul(psS[64:64 + S, c:c + 1],
                                         kT[DK:128, c, bc0:bc0 + S], qL[DK:128, c, bi:bi + 1],
                                         start=True, stop=True)
                    pt = pt_p.tile([128, NDC], BF16, tag="pt")
                    nc.scalar.activation(pt[:], psS[:], EXP, bias=0.0, scale=1.0 / math.sqrt(DK))
                    denE = ps_row.tile([1, NDC], F32, tag="row")
                    denO = ps_row.tile([1, NDC], F32, tag="row")
                    nc.tensor.matmul(denE[:], onesb_c[0:S, :], pt[0:S, :], start=True, stop=True)
                    nc.tensor.matmul(denO[:], onesb_c[64:64 + S, :], pt[64:64 + S, :],
                                     start=True, stop=True)
                    rdE = rden_p.tile([1, NDC], BF16, tag="rden")
                    rdO = rden_p.tile([1, NDC], BF16, tag="rden")
                    with nc.allow_low_precision(reason="softmax denom reciprocal"):
                        nc.vector.reciprocal(rdE[:], denE[:])
                        nc.vector.reciprocal(rdO[:], denO[:])
                    bc = ps_at.tile([128, NDC], F32, tag="at")
                    nc.tensor.matmul(bc[0:S, :], onesb_r[0:1, 0:S], rdE[:],
                                     start=True, stop=True)
                    nc.tensor.matmul(bc[64:64 + S, :], onesb_r[0:1, 0:S], rdO[:],
                                     start=True, stop=True)
                    pn = ptn_p.tile([128, NDC], BF16, tag="ptn")
                    nc.vector.tensor_tensor(out=pn[:], in0=pt[:], in1=bc[:], op=AOP.mult)
                    po = ps_at.tile([128, NDC], F32, tag="at")
                    for c in range(NDC):
                        nc.tensor.matmul(po[0:DK, c:c + 1],
                                         v[0:S, bi, (2 * c) * DK:(2 * c + 1) * DK],
                                         pn[0:S, c:c + 1], start=True, stop=True)
                    for c in range(NDC):
                        nc.tensor.matmul(po[64:128, c:c + 1],
                                         v[64:64 + S, bi, (2 * c + 1) * DK:(2 * c + 2) * DK],
                                         pn[64:64 + S, c:c + 1], start=True, stop=True)
                    nc.vector.tensor_copy(oL[:, :, bi], po[:])
                # tiny Wo: add attention output at last tokens into hL
                for m in range(NDC):
                    wt = wsm.tile([128, NDC, 128], BF16, tag="wsm")
                    nc.sync.dma_start(wt[:], d['wo_w'][li, m].rearrange("p (k c) -> p k c", k=NDC))
                    br = brow_p.tile([1, 128], BF16, tag="brow2")
                    nc.sync.dma_start(br[:], d['wo_b'][li, m])
                    ps = ps_mm.tile([128, NB], F32, tag="mm")
                    nc.tensor.matmul(ps[:], br[:], onesb_r[0:1, 0:NB], start=True, stop=False)
                    for k in range(NDC):
                        nc.tensor.matmul(ps[:], wt[:, k, :], oL[:, k, :],
                                         start=False, stop=(k == NDC - 1))
                    nc.vector.tensor_tensor(out=hL[:, m, :], in0=hL[:, m, :],
                                            in1=ps[:], op=AOP.add)
            # ---- full attention per batch, Wo interleaved per half ----
            if not trim:
                oT0 = sb1.tile([128, NDC, HT], BF16, tag="tagO")
                oT1 = sb1.tile([128, NDC, HT], BF16, tag="tagX")
                oTs = (oT0, oT1)

                def attn_batch(bi):
                    bc0 = bi * S
                    ot = oTs[bi // HB]
                    oc0 = bc0 - (bi // HB) * HT
                    psS = ps_mm.tile([128, NDC * S], F32, tag="mm")
                    for c in range(NDC):
                        nc.tensor.matmul(psS[0:S, c * S:(c + 1) * S],
                                         kT[0:DK, c, bc0:bc0 + S], qT[0:DK, c, bc0:bc0 + S],
                                         start=(c == 0), stop=(c == NDC - 1),
                                         tile_position=(0, 0), skip_group_check=True)
                    for c in range(NDC):
                        nc.tensor.matmul(psS[64:64 + S, c * S:(c + 1) * S],
                                         kT[DK:128, c, bc0:bc0 + S], qT[DK:128, c, bc0:bc0 + S],
                                         start=(c == 0), stop=(c == NDC - 1),
                                         tile_position=(64, 64), skip_group_check=True)
                    pt = pt_p.tile([128, NDC * S], BF16, tag="pt")
                    nc.scalar.activation(pt[:], psS[:], EXP, bias=0.0, scale=1.0 / math.sqrt(DK))
                    denE = ps_row.tile([1, NDC * S], F32, tag="row")
                    denO = ps_row.tile([1, NDC * S], F32, tag="row")
                    nc.tensor.matmul(denE[:], onesb_c[0:S, :], pt[0:S, :], start=True, stop=True)
                    nc.tensor.matmul(denO[:], onesb_c[64:64 + S, :], pt[64:64 + S, :],
                                     start=True, stop=True, tile_position=(64, 0))
                    rdE = rden_p.tile([1, NDC * S], BF16, tag="rden")
                    rdO = rden_p.tile([1, NDC * S], BF16, tag="rden")
                    with nc.allow_low_precision(reason="softmax denom reciprocal"):
                        nc.vector.reciprocal(rdE[:], denE[:])
                        nc.vector.reciprocal(rdO[:], denO[:])
                    bc = ps_at.tile([128, NDC * S], F32, tag="at")
                    nc.tensor.matmul(bc[0:S, :], onesb_r[0:1, 0:S], rdE[:],
                                     start=True, stop=True, tile_position=(0, 0),
                                     skip_group_check=True)
                    nc.tensor.matmul(bc[64:64 + S, :], onesb_r[0:1, 0:S], rdO[:],
                                     start=True, stop=True, tile_position=(0, 64),
                                     skip_group_check=True)
                    pn = ptn_p.tile([128, NDC * S], BF16, tag="ptn")
                    nc.vector.tensor_tensor(out=pn[:], in0=pt[:], in1=bc[:], op=AOP.mult)
                    po = ps_at.tile([128, NDC * S], F32, tag="at")
                    for c in range(NDC):
                        nc.tensor.matmul(po[0:DK, c * S:(c + 1) * S],
                                         v[0:S, bi, (2 * c) * DK:(2 * c + 1) * DK],
                                         pn[0:S, c * S:(c + 1) * S],
                                         start=(c == 0), stop=(c == NDC - 1),
                                         tile_position=(0, 0), skip_group_check=True)
                    for c in range(NDC):
                        nc.tensor.matmul(po[64:128, c * S:(c + 1) * S],
                                         v[64:64 + S, bi, (2 * c + 1) * DK:(2 * c + 2) * DK],
                                         pn[64:64 + S, c * S:(c + 1) * S],
                                         start=(c == 0), stop=(c == NDC - 1),
                                         tile_position=(64, 64), skip_group_check=True)
                    nc.vector.tensor_copy(
                        ot[:, :, oc0:oc0 + S],
                        po[:].rearrange("p (c t) -> p c t", c=NDC))

                def wo_half(hf):
                    hc0 = hf * HT
                    ot = oTs[hf]
                    for m in range(NDC):
                        wt = wsm.tile([128, NDC, 128], BF16, tag="wsm")
                        nc.sync.dma_start(wt[:], d['wo_w'][li, m].rearrange("p (k c) -> p k c", k=NDC))
                        br = brow_p.tile([1, 128], BF16, tag="brow2")
                        nc.sync.dma_start(br[:], d['wo_b'][li, m])
                        ps = ps_mm.tile([128, HT], F32, tag="mm")
                        nc.tensor.matmul(ps[:], br[:], onesb_r[0:1, 0:HT], start=True, stop=False)
                        for k in range(NDC):
                            nc.tensor.matmul(ps[:], wt[:, k, :], ot[:, k, :],
                                             start=False, stop=(k == NDC - 1))
                        nc.vector.tensor_tensor(out=hT[:, m, hc0:hc0 + HT],
                                                in0=hT[:, m, hc0:hc0 + HT],
                                                in1=ps[:], op=AOP.add)

                for bi in range(HB):
                    attn_batch(bi)
                wo_half(0)
                for bi in range(HB, NB):
                    attn_batch(bi)
                wo_half(1)
            # ---- FFN ----
            if not last:
                aT2 = sb1.tile([128, NDC, T], BF16, tag="tagA")
                ln_full(hT, aT2)
                b1T = brow_p.tile([128, NFC], F32, tag="brow")
                nc.sync.dma_start(b1T[:], d['w1_bT'][li])
                if FP8_FFN:
                    aT8 = sb1.tile([128, NDC, T], FP8, tag="tagA8")
                    for c in range(NDC):
                        nc.scalar.activation(aT8[:, c, :], aT2[:, c, :], COPY,
                                             bias=0.0, scale=1.0)
                    fdt = FP8
                else:
                    aT8 = aT2
                    fdt = BF16
                ffq0 = sb1.tile([128, 8, T], fdt, tag="tagQ")
                ffq1 = sb1.tile([128, 8, T], fdt, tag="tagK")
                ffq2 = sb1.tile([128, 8, T], fdt, tag="tagVf")
                ffq3 = sb1.tile([128, 8, T], fdt, tag="tagO")
                ffq = [ffq0, ffq1, ffq2, ffq3]
                for m in range(NFC):
                    if FP8_FFN:
                        wt = wsm.tile([128, NDC // 2, 2, 128], FP8, tag="wsm8")
                        nc.sync.dma_start(wt[:], d['w1_w8'][li, m].rearrange(
                            "p (g j c) -> p g j c", g=NDC // 2, j=2))
                    else:
                        wt = wsm.tile([128, NDC, 128], BF16, tag="wsm")
                        nc.sync.dma_start(wt[:], d['w1_w'][li, m].rearrange("p (k c) -> p k c", k=NDC))
                    for hf in range(2):
                        ps = ps_mm.tile([128, HT], F32, tag="mm")
                        if FP8_FFN:
                            for g in range(NDC // 2):
                                nc.tensor.matmul(ps[:], wt[:, g, :, :],
                                                 aT8[:, 2 * g:2 * g + 2, hf * HT:(hf + 1) * HT],
                                                 start=(g == 0), stop=(g == NDC // 2 - 1),
                                                 perf_mode=DR)
                        else:
                            for k in range(NDC):
                                nc.tensor.matmul(ps[:], wt[:, k, :],
                                                 aT8[:, k, hf * HT:(hf + 1) * HT],
                                                 start=(k == 0), stop=(k == NDC - 1))
                        nc.scalar.activation(ffq[m // 8][:, m % 8, hf * HT:(hf + 1) * HT],
                                             ps[:], RELU, bias=b1T[:, m:m + 1],
                                             scale=(1.0 / 1024.0 if FP8_FFN else 1.0))
                for m in range(NDC):
                    if FP8_FFN:
                        w2t = wst.tile([128, NFC // 2, 2, 128], FP8, tag="wst")
                        nc.sync.dma_start(w2t[:], d['w2_w8'][li, m].rearrange(
                            "p (g j c) -> p g j c", g=NFC // 2, j=2))
                    else:
                        w2t = wst.tile([128, NFC, 128], BF16, tag="wst")
                        nc.sync.dma_start(w2t[:], d['w2_w'][li, m].rearrange("p (k c) -> p k c", k=NFC))
                    br = brow_p.tile([1, 128], BF16, tag="brow2")
                    nc.sync.dma_start(br[:], d['w2_b'][li, m])
                    for hf in range(2):
                        hc0 = hf * HT
                        ps = ps_mm.tile([128, HT], F32, tag="mm")
                        nc.tensor.matmul(ps[:], br[:], onesb_r[0:1, 0:HT], start=True, stop=False)
                        if FP8_FFN:
                            for g in range(NFC // 2):
                                nc.tensor.matmul(ps[:], w2t[:, g, :, :],
                                                 ffq[(2 * g) // 8][:, (2 * g) % 8:(2 * g) % 8 + 2,
                                                                   hc0:hc0 + HT],
                                                 start=False, stop=(g == NFC // 2 - 1),
                                                 perf_mode=DR)
                        else:
                            for k in range(NFC):
                                nc.tensor.matmul(ps[:], w2t[:, k, :],
                                                 ffq[k // 8][:, k % 8, hc0:hc0 + HT],
                                                 start=False, stop=(k == NFC - 1))
                        nc.vector.tensor_tensor(out=hT[:, m, hc0:hc0 + HT],
                                                in0=hT[:, m, hc0:hc0 + HT],
                                                in1=ps[:], op=AOP.add)
            else:
                # last layer: FFN only for the last token of each batch
                if not trim:
                    hL = sb1.tile([128, NDC, NB], F32R, tag="hL")
                    for c in range(NDC):
                        nc.vector.tensor_copy(
                            hL[:, c, :],
                            hT[:, c, :].rearrange("p (b s) -> p b s", s=S)[:, :, S - 1])
                aL = sb1.tile([128, NDC, NB], BF16, tag="aL")
                ln_half(hL, 0, NB, aL, NDC)
                b1T = brow_p.tile([128, NFC], F32, tag="brow")
                nc.sync.dma_start(b1T[:], d['w1_bT'][li])
                if FP8_FFN:
                    aL8 = sb1.tile([128, NDC, NB], FP8, tag="aL8")
                    nc.scalar.activation(aL8[:], aL[:], COPY, bias=0.0, scale=1.0)
                    fdt = FP8
                else:
                    aL8 = aL
                    fdt = BF16
                ffL = sb1.tile([128, NFC, NB], fdt, tag="ffL")
                for m in range(NFC):
                    if FP8_FFN:
                        wt = wsm.tile([128, NDC // 2, 2, 128], FP8, tag="wsm8")
                        nc.sync.dma_start(wt[:], d['w1_w8'][li, m].rearrange(
                            "p (g j c) -> p g j c", g=NDC // 2, j=2))
                    else:
                        wt = wsm.tile([128, NDC, 128], BF16, tag="wsm")
                        nc.sync.dma_start(wt[:], d['w1_w'][li, m].rearrange("p (k c) -> p k c", k=NDC))
                    ps = ps_mm.tile([128, NB], F32, tag="mm")
                    if FP8_FFN:
                        for g in range(NDC // 2):
                            nc.tensor.matmul(ps[:], wt[:, g, :, :],
                                             aL8[:, 2 * g:2 * g + 2, :],
                                             start=(g == 0), stop=(g == NDC // 2 - 1),
                                             perf_mode=DR)
                    else:
                        for k in range(NDC):
                            nc.tensor.matmul(ps[:], wt[:, k, :], aL8[:, k, :],
                                             start=(k == 0), stop=(k == NDC - 1))
                    nc.scalar.activation(ffL[:, m, :], ps[:], RELU,
                                         bias=b1T[:, m:m + 1],
                                         scale=(1.0 / 1024.0 if FP8_FFN else 1.0))
                for m in range(NDC):
                    if FP8_FFN:
                        w2t = wst.tile([128, NFC // 2, 2, 128], FP8, tag="wst")
                        nc.sync.dma_start(w2t[:], d['w2_w8'][li, m].rearrange(
                            "p (g j c) -> p g j c", g=NFC // 2, j=2))
                    else:
                        w2t = wst.tile([128, NFC, 128], BF16, tag="wst")
                        nc.sync.dma_start(w2t[:], d['w2_w'][li, m].rearrange("p (k c) -> p k c", k=NFC))
                    br = brow_p.tile([1, 128], BF16, tag="brow2")
                    nc.sync.dma_start(br[:], d['w2_b'][li, m])
                    ps = ps_mm.tile([128, NB], F32, tag="mm")
                    nc.tensor.matmul(ps[:], br[:], onesb_r[0:1, 0:NB], start=True, stop=False)
                    if FP8_FFN:
                        for g in range(NFC // 2):
                            nc.tensor.matmul(ps[:], w2t[:, g, :, :],
                                             ffL[:, 2 * g:2 * g + 2, :],
                                             start=False, stop=(g == NFC // 2 - 1),
                                             perf_mode=DR)
                    else:
                        for k in range(NFC):
                            nc.tensor.matmul(ps[:], w2t[:, k, :], ffL[:, k, :],
                                             start=False, stop=(k == NFC - 1))
                    nc.vector.tensor_tensor(out=hL[:, m, :], in0=hL[:, m, :],
                                            in1=ps[:], op=AOP.add)

        # ---------------- head ----------------
        if n_layers == L:
            src_pool = hL
        else:
            src_pool = sb1.tile([128, NDC, NB], F32R, tag="hL")
            for c in range(NDC):
                nc.vector.tensor_copy(
                    src_pool[:, c, :],
                    hT[:, c, :].rearrange("p (b s) -> p b s", s=S)[:, :, S - 1])
        pL = sb1.tile([128, NDC, NB], BF16, tag="pL")
        ln_half(src_pool, 0, NB, pL, NDC)
        cbT = brow_p.tile([128, NDC], F32, tag="brow")
        nc.sync.dma_start(cbT[:], d['cf_bT'][:])
        z1 = sb1.tile([128, NDC, NB], BF16, tag="z1")
        for m in range(NDC):
            wt = wsm.tile([128, NDC, 128], BF16, tag="wsm")
            nc.sync.dma_start(wt[:], d['cf_w'][m].rearrange("p (k c) -> p k c", k=NDC))
            ps = ps_mm.tile([128, NB], F32, tag="mm")
            for k in range(NDC):
                nc.tensor.matmul(ps[:], wt[:, k, :], pL[:, k, :],
                                 start=(k == 0), stop=(k == NDC - 1))
            nc.scalar.activation(z1[:, m, :], ps[:], RELU, bias=cbT[:, m:m + 1], scale=1.0)
        fwt = sb1.tile([128, NDC, NCLS], BF16, tag="fwt")
        nc.sync.dma_start(fwt[:], d['fc_w'].rearrange("p (k c) -> p k c", k=NDC))
        fb = brow_p.tile([NCLS, 1], F32, tag="brow2")
        nc.sync.dma_start(fb[:], d['fc_b'][:])
        ps = ps_mm.tile([NCLS, NB], F32, tag="mm")
        for k in range(NDC):
            nc.tensor.matmul(ps[:], fwt[:, k, :], z1[:, k, :],
                             start=(k == 0), stop=(k == NDC - 1))
        osb = sb1.tile([NCLS, NB], F32, tag="osb")
        nc.vector.tensor_scalar_add(osb[:], ps[:], fb[:])
        nc.sync.dma_start(out[:], osb[:])


def _prep_weights(inputs, n_layers=L):
    f64 = np.float64

    def prep_lhsT(W):
        # W [K, M] -> [M/128, 128, (K/128)*128] : tile[p, k*128+c] = W[k*128+p, mb*128+c]
        K, M = W.shape
        nk, nm = K // 128, M // 128
        return np.ascontiguousarray(
            W.reshape(nk, 128, nm, 128).transpose(2, 1, 0, 3).reshape(nm, 128, nk * 128)
        ).astype(BF16NP)

    emb = inputs['embed_w'].astype(f64)          # [1200, 1024]
    pos = np.arange(S, dtype=f64)[:, None]
    div = np.exp(np.arange(0, D, 2, dtype=np.float32).astype(f64) * (-math.log(10000.0) / D))
    pe = np.zeros((S, D), f64)
    pe[:, 0::2] = np.sin(pos * div)
    pe[:, 1::2] = np.cos(pos * div)
    Wp = np.zeros((NIP, D), f64)
    Wp[:NI] = emb
    Wp[NI:NI + S] = pe
    g = {}
    g['emb_w'] = prep_lhsT(Wp)

    ln_g = inputs['ln_g'].astype(f64); ln_b = inputs['ln_b'].astype(f64)
    aw = inputs['attn_w'].astype(f64); ab = inputs['attn_b'].astype(f64)
    fw1 = inputs['ff_w1'].astype(f64); fb1 = inputs['ff_b1'].astype(f64)
    fw2 = inputs['ff_w2'].astype(f64); fb2 = inputs['ff_b2'].astype(f64)

    qkv_w = np.zeros((L, 3, NDC, 128, NDC * 128), BF16NP)
    qkv_bT = np.zeros((L, 128, 3 * NDC), np.float32)
    wo_w = np.zeros((L, NDC, 128, NDC * 128), BF16NP)
    wo_b = np.zeros((L, NDC, 1, 128), BF16NP)
    wdt = FP8NP if FP8_FFN else BF16NP
    w1_w = np.zeros((L, NFC, 128, NDC * 128), wdt)
    w1_bT = np.zeros((L, 128, NFC), np.float32)
    w2_w = np.zeros((L, NDC, 128, NFC * 128), wdt)
    w2_b = np.zeros((L, NDC, 1, 128), BF16NP)

    def prep_dr(W, scale):
        # W [K, M] -> pair-interleaved DoubleRow layout
        # tile[p, g*256 + j*128 + c] = scale * W[(2g+j)*128 + p, m*128 + c]
        K, M = W.shape
        ng, nm = K // 256, M // 128
        arr = (W * scale).reshape(ng, 2, 128, nm, 128)
        return np.ascontiguousarray(
            arr.transpose(3, 2, 0, 1, 4).reshape(nm, 128, ng * 256)).astype(FP8NP)

    for i in range(n_layers):
        g1, b1 = ln_g[i, 0][:, None], ln_b[i, 0]
        for mat in range(3):
            We = g1 * aw[i, mat]
            be = ab[i, mat] + b1 @ aw[i, mat]
            qkv_w[i, mat] = prep_lhsT(We)
            if mat == 2:
                bv = be  # v bias folded into wo_b (softmax rows sum to 1)
            else:
                qkv_bT[i, :, mat * NDC:(mat + 1) * NDC] = be.reshape(NDC, 128).T
        wo_w[i] = prep_lhsT(aw[i, 3])
        wo_be = ab[i, 3] + bv @ aw[i, 3]
        wo_b[i] = wo_be.reshape(NDC, 1, 128).astype(BF16NP)
        g2, b2 = ln_g[i, 1][:, None], ln_b[i, 1]
        W1e = g2 * fw1[i]
        b1e = fb1[i] + b2 @ fw1[i]
        if FP8_FFN:
            # FFN1 weights x64 for fp8 range; relu scale 1/1024 makes
            # ffq = relu_out / 16; FFN2 weights x16 restores the product.
            w1_w[i] = prep_dr(W1e, 64.0)
            w1_bT[i] = (b1e / 16.0).reshape(NFC, 128).T
            w2_w[i] = prep_dr(fw2[i], 16.0)
        else:
            w1_w[i] = prep_lhsT(W1e)
            w1_bT[i] = b1e.reshape(NFC, 128).T
            w2_w[i] = prep_lhsT(fw2[i])
        w2_b[i] = fb2[i].reshape(NDC, 1, 128).astype(BF16NP)

    g['qkv_w'] = qkv_w; g['qkv_bT'] = qkv_bT
    g['wo_w'] = wo_w; g['wo_b'] = wo_b
    g['w1_bT'] = w1_bT; g['w2_b'] = w2_b
    if FP8_FFN:
        g['w1_w8'] = w1_w; g['w2_w8'] = w2_w
    else:
        g['w1_w'] = w1_w; g['w2_w'] = w2_w

    inv = 1.0 / math.sqrt(1.0 + 1e-5)
    fin_g = inputs['fin_g'].astype(f64); fin_b = inputs['fin_b'].astype(f64)
    A1 = fin_g * inv * inputs['cf_bn_g'].astype(f64)
    C1 = fin_b * inv * inputs['cf_bn_g'].astype(f64) + inputs['cf_bn_b'].astype(f64)
    cfw = inputs['cf_w'].astype(f64)
    cf_we = A1[:, None] * cfw
    cf_be = inputs['cf_b'].astype(f64) + C1 @ cfw
    g['cf_w'] = prep_lhsT(cf_we)
    g['cf_bT'] = cf_be.reshape(NDC, 128).T.astype(np.float32)
    A2 = inv * inputs['fc_bn_g'].astype(f64)
    C2 = inputs['fc_bn_b'].astype(f64)
    fcw = inputs['fc_w'].astype(f64)
    fc_we = A2[:, None] * fcw
    fc_be = inputs['fc_b'].astype(f64) + C2 @ fcw
    g['fc_w'] = np.ascontiguousarray(
        fc_we.reshape(NDC, 128, NCLS).transpose(1, 0, 2).reshape(128, NDC * NCLS)
    ).astype(BF16NP)
    g['fc_b'] = fc_be.reshape(NCLS, 1).astype(np.float32)
    g['ones'] = np.ones((128, 512), np.float32)
    g['onesb'] = np.ones((128, 512), BF16NP)
    g['ident'] = np.eye(128, dtype=BF16NP)
    return g


def _run_timed(nc, in_maps, n_iters=10):
    """Mirror bass2jax.run_bass_via_pjrt (no donation), time steady-state execs."""
    import time
    import jax
    import numpy as _np
    from jax.experimental.shard_map import shard_map
    from jax.sharding import Mesh, PartitionSpec, NamedSharding
    from concourse import bass2jax as b2j
    from concourse import mybir as _mb

    b2j.install_neuronx_cc_hook()
    n_cores = len(in_maps)
    partition_name = nc.partition_id_tensor.name if nc.partition_id_tensor else None
    in_names, out_names, out_avals, zero_outs = [], [], [], []
    for alloc in nc.m.functions[0].allocations:
        if not isinstance(alloc, _mb.MemoryLocationSet):
            continue
        name = alloc.memorylocations[0].name
        if alloc.kind == "ExternalInput":
            if name != partition_name:
                in_names.append(name)
        elif alloc.kind == "ExternalOutput":
            shape = tuple(alloc.tensor_shape)
            dtype = _mb.dt.np(alloc.dtype)
            out_names.append(name)
            out_avals.append(jax.core.ShapedArray(shape, dtype))
            zero_outs.append(_np.zeros(shape, dtype))
    n_params = len(in_names)
    all_in_names = list(in_names) + list(out_names)
    if partition_name is not None:
        all_in_names.append(partition_name)

    def _body(*args):
        operands = list(args)
        if partition_name is not None:
            operands.append(b2j.partition_id_tensor())
        outs = b2j._bass_exec_p.bind(
            *operands,
            out_avals=tuple(out_avals),
            in_names=tuple(all_in_names),
            out_names=tuple(out_names),
            lowering_input_output_aliases=(),
            sim_require_finite=True,
            sim_require_nnan=True,
            nc=nc,
        )
        return tuple(outs)

    devices = jax.devices()[:n_cores]
    mesh = Mesh(_np.asarray(devices), ("core",))
    spec = PartitionSpec("core")
    sharded = jax.jit(shard_map(
        _body, mesh=mesh, in_specs=(spec,) * (n_params + len(out_names)),
        out_specs=(spec,) * len(out_names), check_rep=False))
    sh = NamedSharding(mesh, spec)
    concat_in = [
        jax.device_put(_np.concatenate([_np.asarray(m[name]) for m in in_maps], axis=0), sh)
        for name in in_names
    ]
    concat_zeros = [
        jax.device_put(_np.zeros((n_cores * z.shape[0], *z.shape[1:]), z.dtype), sh)
        for z in zero_outs
    ]
    outs = sharded(*concat_in, *concat_zeros)
    jax.block_until_ready(outs)
    t0 = time.time()
    for _ in range(n_iters):
        outs = sharded(*concat_in, *concat_zeros)
    jax.block_until_ready(outs)
    t1 = time.time()
    per_call_ns = (t1 - t0) / n_iters * 1e9
    results = [
        {name: _np.asarray(outs[i]).reshape(n_cores, *out_avals[i].shape)[c]
         for i, name in enumerate(out_names)}
        for c in range(n_cores)
    ]
    return results, per_call_ns


def _make_in_maps(inputs, g):
    x = inputs['x']
    xr = np.asarray(x).reshape(B, S, NI)
    in_maps = []
    for ci in range(NCORES):
        xc = xr[ci * NB:(ci + 1) * NB].astype(np.float64)  # [16, 50, 1200]
        xa = np.zeros((NB, S, NIP), np.float32)
        xa[:, :, :NI] = xc
        xa[np.arange(NB)[:, None], np.arange(S)[None, :], NI + np.arange(S)[None, :]] = 1.0
        # xT [NKI, 128, T]: feature-major, tokens (b, s)
        xT = np.ascontiguousarray(
            xa.reshape(T, NIP).T.reshape(NKI, 128, T)).astype(BF16NP)
        m = dict(g)
        m['xT'] = xT
        in_maps.append(m)
    return in_maps


def kernel(**inputs):
    global LAST_EXEC_NS
    n_layers = int(inputs.pop('_n_layers', L))
    if n_layers not in _CACHE:
        _CACHE[n_layers] = _build(n_layers)
    nc = _CACHE[n_layers]
    g = _prep_weights(inputs, n_layers)
    in_maps = _make_in_maps(inputs, g)

    results, per_call_ns = _run_timed(nc, in_maps)
    LAST_EXEC_NS = int(per_call_ns)
    outs = [r['out'].T for r in results]   # each [NB, NCLS]
    return np.concatenate(outs, axis=0).astype(np.float32)



# revision 2
# speedup vs baseline: 4.1548x; 4.1548x over previous
import sys
sys.path.insert(0, '/opt/trn_rl_repo')
import numpy as np
import math

import concourse.bass as bass
import concourse.mybir as mybir
import concourse.tile as tile
from concourse import bacc
from concourse.bass_utils import run_bass_kernel_spmd

import ml_dtypes
BF16NP = ml_dtypes.bfloat16

# Problem dims
B, SL, CH, HZ = 128, 5000, 12, 100
L, D, DFF, H, NCLS = 5, 1024, 4096, 16, 71
NI = CH * HZ          # 1200
S = SL // HZ          # 50
NCORES = 8
NB = B // NCORES      # 16 batches per core
T = NB * S            # 800 tokens per core
NIP = 1280            # padded input-feature dim
NKI = NIP // 128      # 10 input k-chunks
DK = D // H           # 64
NDC = D // 128        # 8 d-chunks
NFC = DFF // 128      # 32 ff-chunks
HB = NB // 2          # 8 batches per half
HT = HB * S           # 400 tokens per half

F32R = mybir.dt.float32r
F32 = mybir.dt.float32
BF16 = mybir.dt.bfloat16
EXP = mybir.ActivationFunctionType.Exp
RELU = mybir.ActivationFunctionType.Relu
COPY = mybir.ActivationFunctionType.Copy
AOP = mybir.AluOpType

TRACE = False
LAST_EXEC_NS = None
_CACHE = {}
FP8_FFN = False
TRIM_L5 = False
FP8 = mybir.dt.float8e4
FP8NP = ml_dtypes.float8_e4m3
DR = mybir.MatmulPerfMode.DoubleRow


def _build(n_layers=L):
    nc = bacc.Bacc(None)
    d = {}
    d['xT'] = nc.dram_tensor("xT", [NKI, 128, T], BF16, kind="ExternalInput")
    d['ones'] = nc.dram_tensor("ones", [128, 512], F32R, kind="ExternalInput")
    d['onesb'] = nc.dram_tensor("onesb", [128, 512], BF16, kind="ExternalInput")
    d['ident'] = nc.dram_tensor("ident", [128, 128], BF16, kind="ExternalInput")
    d['emb_w'] = nc.dram_tensor("emb_w", [NDC, 128, NKI * 128], BF16, kind="ExternalInput")
    d['qkv_w'] = nc.dram_tensor("qkv_w", [L, 3, NDC, 128, NDC * 128], BF16, kind="ExternalInput")
    d['qkv_bT'] = nc.dram_tensor("qkv_bT", [L, 128, 3 * NDC], F32, kind="ExternalInput")
    d['wo_w'] = nc.dram_tensor("wo_w", [L, NDC, 128, NDC * 128], BF16, kind="ExternalInput")
    d['wo_b'] = nc.dram_tensor("wo_b", [L, NDC, 1, 128], BF16, kind="ExternalInput")
    d['w1_bT'] = nc.dram_tensor("w1_bT", [L, 128, NFC], F32, kind="ExternalInput")
    d['w2_b'] = nc.dram_tensor("w2_b", [L, NDC, 1, 128], BF16, kind="ExternalInput")
    if FP8_FFN:
        d['w1_w8'] = nc.dram_tensor("w1_w8", [L, NFC, 128, NDC * 128], FP8, kind="ExternalInput")
        d['w2_w8'] = nc.dram_tensor("w2_w8", [L, NDC, 128, NFC * 128], FP8, kind="ExternalInput")
    else:
        d['w1_w'] = nc.dram_tensor("w1_w", [L, NFC, 128, NDC * 128], BF16, kind="ExternalInput")
        d['w2_w'] = nc.dram_tensor("w2_w", [L, NDC, 128, NFC * 128], BF16, kind="ExternalInput")
    d['cf_w'] = nc.dram_tensor("cf_w", [NDC, 128, NDC * 128], BF16, kind="ExternalInput")
    d['cf_bT'] = nc.dram_tensor("cf_bT", [128, NDC], F32, kind="ExternalInput")
    d['fc_w'] = nc.dram_tensor("fc_w", [128, NDC * NCLS], BF16, kind="ExternalInput")
    d['fc_b'] = nc.dram_tensor("fc_b", [NCLS, 1], F32, kind="ExternalInput")
    out = nc.dram_tensor("out", [NCLS, NB], F32, kind="ExternalOutput")

    with tile.TileContext(nc) as tc:
        _emit(nc, tc, d, out, n_layers)
    nc.compile()
    return nc


def _emit(nc, tc, d, out, n_layers):
    import contextlib
    ctx = contextlib.ExitStack()
    with ctx:
        sb1 = ctx.enter_context(tc.tile_pool(name="sb1", bufs=1))
        sq_p = ctx.enter_context(tc.tile_pool(name="sqp", bufs=2))
        wsm = ctx.enter_context(tc.tile_pool(name="wsm", bufs=8))
        wst = ctx.enter_context(tc.tile_pool(name="wst", bufs=3))
        rows = ctx.enter_context(tc.tile_pool(name="rows", bufs=4))
        rden_p = ctx.enter_context(tc.tile_pool(name="rden", bufs=2))
        brow_p = ctx.enter_context(tc.tile_pool(name="brow", bufs=2))
        pt_p = ctx.enter_context(tc.tile_pool(name="ptp", bufs=2))
        ptn_p = ctx.enter_context(tc.tile_pool(name="ptnp", bufs=2))
        ps_mm = ctx.enter_context(tc.tile_pool(name="psmm", bufs=2, space="PSUM"))
        ps_at = ctx.enter_context(tc.tile_pool(name="psat", bufs=4, space="PSUM"))
        ps_row = ctx.enter_context(tc.tile_pool(name="psrow", bufs=2, space="PSUM"))

        # persistent tiles
        hT = sb1.tile([128, NDC, T], F32R, tag="hT")
        ones_c = sb1.tile([128, 1], F32R, tag="ones_c")
        ones_r = sb1.tile([1, 512], F32R, tag="ones_r")
        onesb_c = sb1.tile([128, 1], BF16, tag="onesb_c")
        onesb_r = sb1.tile([1, 512], BF16, tag="onesb_r")
        ident = sb1.tile([128, 128], BF16, tag="ident")
        nc.sync.dma_start(ones_c[:], d['ones'][:, 0:1])
        nc.sync.dma_start(ones_r[:], d['ones'][0:1, :])
        nc.sync.dma_start(onesb_c[:], d['onesb'][:, 0:1])
        nc.sync.dma_start(onesb_r[:], d['onesb'][0:1, :])
        nc.sync.dma_start(ident[:], d['ident'][:, :])

        def ln_half(src, hcol0, ncols, dst, nch):
            """LN over feature dim (nch*128) of src[:, :, hcol0:hcol0+ncols] ->
            dst[:, :, 0:ncols] (bf16). src is [128, nch, *] fp32r."""
            Dn = float(nch * 128)
            cm = 1.0 / Dn
            cv2 = 1.0 / (Dn - 1.0)
            cv1 = -1.0 / (Dn * (Dn - 1.0))
            s1 = ps_row.tile([1, ncols], F32, tag="row")
            s2 = ps_row.tile([1, ncols], F32, tag="row")
            for c in range(nch):
                sq = sq_p.tile([128, ncols], F32R, tag="sq")
                nc.scalar.square(sq[:], src[:, c, hcol0:hcol0 + ncols])
                nc.tensor.matmul(s1[:], ones_c[:], src[:, c, hcol0:hcol0 + ncols],
                                 start=(c == 0), stop=(c == nch - 1))
                nc.tensor.matmul(s2[:], ones_c[:], sq[:],
                                 start=(c == 0), stop=(c == nch - 1))
            m_row = rows.tile([1, ncols], F32R, tag="rowsb")
            t1 = rows.tile([1, ncols], F32, tag="rowsb")
            t2 = rows.tile([1, ncols], F32, tag="rowsb")
            nc.vector.tensor_scalar_mul(m_row[:], s1[:], cm)
            nc.scalar.square(t1[:], s1[:])
            nc.vector.tensor_scalar_mul(t1[:], t1[:], cv1)
            nc.vector.tensor_scalar_mul(t2[:], s2[:], cv2)
            nc.vector.tensor_tensor(out=t1[:], in0=t1[:], in1=t2[:], op=AOP.add)
            nc.scalar.sqrt(t1[:], t1[:])
            nc.vector.tensor_scalar_add(t1[:], t1[:], 1e-6)
            r_row = rows.tile([1, ncols], F32R, tag="rowsb")
            with nc.allow_low_precision(reason="fp32r rounding of 1/(std+eps)"):
                nc.vector.reciprocal(r_row[:], t1[:])
            Mb = ps_at.tile([128, ncols], F32, tag="at")
            Rb = ps_at.tile([128, ncols], F32, tag="at")
            nc.tensor.matmul(Mb[:], ones_r[0:1, 0:128], m_row[:], start=True, stop=True)
            nc.tensor.matmul(Rb[:], ones_r[0:1, 0:128], r_row[:], start=True, stop=True)
            for c in range(nch):
                dslice = dst[:, c, hcol0:hcol0 + ncols]
                nc.vector.tensor_tensor(out=dslice, in0=src[:, c, hcol0:hcol0 + ncols],
                                        in1=Mb[:], op=AOP.subtract)
                nc.vector.tensor_tensor(out=dslice, in0=dslice,
                                        in1=Rb[:], op=AOP.mult)

        def ln_full(src, dst, nch=NDC, ncols_tot=T):
            for hf in range(ncols_tot // HT):
                ln_half(src, hf * HT, HT, dst, nch)

        # ---------------- embed ----------------
        xt = sb1.tile([128, NKI, T], BF16, tag="tagX")
        nc.sync.dma_start(xt[:], d['xT'].rearrange("k p t -> p k t"))
        for m in range(NDC):
            wt = wst.tile([128, NKI, 128], BF16, tag="wste")
            nc.sync.dma_start(wt[:], d['emb_w'][m].rearrange("p (k c) -> p k c", k=NKI))
            for hf in range(2):
                ps = ps_mm.tile([128, HT], F32, tag="mm")
                for k in range(NKI):
                    nc.tensor.matmul(ps[:], wt[:, k, :], xt[:, k, hf * HT:(hf + 1) * HT],
                                     start=(k == 0), stop=(k == NKI - 1))
                nc.vector.tensor_copy(hT[:, m, hf * HT:(hf + 1) * HT], ps[:])

        # ---------------- layers ----------------
        for li in range(n_layers):
            last = (li == n_layers - 1) and (n_layers == L)
            trim = last and TRIM_L5
            # ---- LN1 over full T ----
            aT = sb1.tile([128, NDC, T], BF16, tag="tagA")
            ln_full(hT, aT)
            # ---- Q, K (single weight pass, both halves) ----
            kT = sb1.tile([128, NDC, T], BF16, tag="tagK")
            bT = brow_p.tile([128, 2 * NDC], F32, tag="brow")
            nc.sync.dma_start(bT[:], d['qkv_bT'][li, :, 0:2 * NDC])
            if not trim:
                qT = sb1.tile([128, NDC, T], BF16, tag="tagQ")
                mats = ((0, qT), (1, kT))
            else:
                mats = ((1, kT),)
            for mat, dst in mats:
                for m in range(NDC):
                    wt = wsm.tile([128, NDC, 128], BF16, tag="wsm")
                    nc.sync.dma_start(wt[:], d['qkv_w'][li, mat, m].rearrange("p (k c) -> p k c", k=NDC))
                    for hf in range(2):
                        ps = ps_mm.tile([128, HT], F32, tag="mm")
                        for k in range(NDC):
                            nc.tensor.matmul(ps[:], wt[:, k, :], aT[:, k, hf * HT:(hf + 1) * HT],
                                             start=(k == 0), stop=(k == NDC - 1))
                        nc.vector.tensor_scalar_add(
                            dst[:, m, hf * HT:(hf + 1) * HT], ps[:],
                            bT[:, mat * NDC + m:mat * NDC + m + 1])
            if trim:
                # Q only for the last token of each batch
                aL16 = sb1.tile([128, NDC, NB], BF16, tag="aL16")
                nc.vector.tensor_copy(
                    aL16[:],
                    aT[:].rearrange("p c (b s) -> p c b s", s=S)[:, :, :, S - 1])
                qL = sb1.tile([128, NDC, NB], BF16, tag="tagQ")
                for m in range(NDC):
                    wt = wsm.tile([128, NDC, 128], BF16, tag="wsm")
                    nc.sync.dma_start(wt[:], d['qkv_w'][li, 0, m].rearrange("p (k c) -> p k c", k=NDC))
                    ps = ps_mm.tile([128, NB], F32, tag="mm")
                    for k in range(NDC):
                        nc.tensor.matmul(ps[:], wt[:, k, :], aL16[:, k, :],
                                         start=(k == 0), stop=(k == NDC - 1))
                    nc.vector.tensor_scalar_add(qL[:, m, :], ps[:], bT[:, m:m + 1])
            # ---- V: feature-major GEMM then per-batch transposes ----
            vTf = sb1.tile([128, NDC, T], BF16, tag="tagVf")
            for m in range(NDC):
                wt = wsm.tile([128, NDC, 128], BF16, tag="wsm")
                nc.sync.dma_start(wt[:], d['qkv_w'][li, 2, m].rearrange("p (k c) -> p k c", k=NDC))
                for hf in range(2):
                    ps = ps_mm.tile([128, HT], F32, tag="mm")
                    for k in range(NDC):
                        nc.tensor.matmul(ps[:], wt[:, k, :], aT[:, k, hf * HT:(hf + 1) * HT],
                                         start=(k == 0), stop=(k == NDC - 1))
                    nc.vector.tensor_copy(vTf[:, m, hf * HT:(hf + 1) * HT], ps[:])
            # v token-major, replicated: rows 0-49 and 64-113 (bf16)
            v = sb1.tile([128, NB, D], BF16, tag="tagV")
            for bi in range(NB):
                bc0 = bi * S
                pv = ps_at.tile([128, D], BF16, tag="at")
                for c in range(NDC):
                    nc.tensor.transpose(pv[0:S, c * 128:(c + 1) * 128],
                                        vTf[:, c, bc0:bc0 + S], ident[:],
                                        tile_position=(0, 0))
                nc.vector.tensor_copy(v[0:S, bi, :], pv[0:S, :])
                # odd-head replica rows 64-113 via partition-shift DMA
                nc.sync.dma_start(v[64:64 + S, bi, :], v[0:S, bi, :])
            if trim:
                # extract last-token residual before adding attention output
                hL = sb1.tile([128, NDC, NB], F32R, tag="hL")
                for c in range(NDC):
                    nc.vector.tensor_copy(
                        hL[:, c, :],
                        hT[:, c, :].rearrange("p (b s) -> p b s", s=S)[:, :, S - 1])
                # tiny attention: 1 query per batch
                oL = sb1.tile([128, NDC, NB], BF16, tag="tagO")
                for bi in range(NB):
                    bc0 = bi * S
                    psS = ps_at.tile([128, NDC], F32, tag="at")
                    for c in range(NDC):
                        nc.tensor.matmul(psS[0:S, c:c + 1],
                                         kT[0:DK, c, bc0:bc0 + S], qL[0:DK, c, bi:bi + 1],
                                         start=True, stop=True)
                    for c in range(NDC):
                        nc.tensor.matmul(psS[64:64 + S, c:c + 1],
                                         kT[DK:128, c, bc0:bc0 + S], qL[DK:128, c, bi:bi + 1],
                                         start=True, stop=True)
                    pt = pt_p.tile([128, NDC], BF16, tag="pt")
                    nc.scalar.activation(pt[:], psS[:], EXP, bias=0.0, scale=1.0 / math.sqrt(DK))
                    denE = ps_row.tile([1, NDC], F32, tag="row")
                    denO = ps_row.tile([1, NDC], F32, tag="row")
                    nc.tensor.matmul(denE[:], onesb_c[0:S, :], pt[0:S, :], start=True, stop=True)
                    nc.tensor.matmul(denO[:], onesb_c[64:64 + S, :], pt[64:64 + S, :],
                                     start=True, stop=True)
                    rdE = rden_p.tile([1, NDC], BF16, tag="rden")
                    rdO = rden_p.tile([1, NDC], BF16, tag="rden")
                    with nc.allow_low_precision(reason="softmax denom reciprocal"):
                        nc.vector.reciprocal(rdE[:], denE[:])
                        nc.vector.reciprocal(rdO[:], denO[:])
                    bc = ps_at.tile([128, NDC], F32, tag="at")
                    nc.tensor.matmul(bc[0:S, :], onesb_r[0:1, 0:S], rdE[:],
                                     start=True, stop=True)
                    nc.tensor.matmul(bc[64:64 + S, :], onesb_r[0:1, 0:S], rdO[:],
                                     start=True, stop=True)
                    pn = ptn_p.tile([128, NDC], BF16, tag="ptn")
                    nc.vector.tensor_tensor(out=pn[:], in0=pt[:], in1=bc[:], op=AOP.mult)
                    po = ps_at.tile([128, NDC], F32, tag="at")
                    for c in range(NDC):
                        nc.tensor.matmul(po[0:DK, c:c + 1],
                                         v[0:S, bi, (2 * c) * DK:(2 * c + 1) * DK],
                                         pn[0:S, c:c + 1], start=True, stop=True)
                    for c in range(NDC):
                        nc.tensor.matmul(po[64:128, c:c + 1],
                                         v[64:64 + S, bi, (2 * c + 1) * DK:(2 * c + 2) * DK],
                                         pn[64:64 + S, c:c + 1], start=True, stop=True)
                    nc.vector.tensor_copy(oL[:, :, bi], po[:])
                # tiny Wo: add attention output at last tokens into hL
                for m in range(NDC):
                    wt = wsm.tile([128, NDC, 128], BF16, tag="wsm")
                    nc.sync.dma_start(wt[:], d['wo_w'][li, m].rearrange("p (k c) -> p k c", k=NDC))
                    br = brow_p.tile([1, 128], BF16, tag="brow2")
                    nc.sync.dma_start(br[:], d['wo_b'][li, m])
                    ps = ps_mm.tile([128, NB], F32, tag="mm")
                    nc.tensor.matmul(ps[:], br[:], onesb_r[0:1, 0:NB], start=True, stop=False)
                    for k in range(NDC):
                        nc.tensor.matmul(ps[:], wt[:, k, :], oL[:, k, :],
                                         start=False, stop=(k == NDC - 1))
                    nc.vector.tensor_tensor(out=hL[:, m, :], in0=hL[:, m, :],
                                            in1=ps[:], op=AOP.add)
            # ---- full attention per batch, Wo interleaved per half ----
            if not trim:
                oT0 = sb1.tile([128, NDC, HT], BF16, tag="tagO")
                oT1 = sb1.tile([128, NDC, HT], BF16, tag="tagX")
                oTs = (oT0, oT1)

                def attn_batch(bi):
                    bc0 = bi * S
                    ot = oTs[bi // HB]
                    oc0 = bc0 - (bi // HB) * HT
                    psS = ps_mm.tile([128, NDC * S], F32, tag="mm")
                    for c in range(NDC):
                        nc.tensor.matmul(psS[0:S, c * S:(c + 1) * S],
                                         kT[0:DK, c, bc0:bc0 + S], qT[0:DK, c, bc0:bc0 + S],
                                         start=(c == 0), stop=(c == NDC - 1),
                                         tile_position=(0, 0), skip_group_check=True)
                    for c in range(NDC):
                        nc.tensor.matmul(psS[64:64 + S, c * S:(c + 1) * S],
                                         kT[DK:128, c, bc0:bc0 + S], qT[DK:128, c, bc0:bc0 + S],
                                         start=(c == 0), stop=(c == NDC - 1),
                                         tile_position=(64, 64), skip_group_check=True)
                    pt = pt_p.tile([128, NDC * S], BF16, tag="pt")
                    nc.scalar.activation(pt[:], psS[:], EXP, bias=0.0, scale=1.0 / math.sqrt(DK))
                    denE = ps_row.tile([1, NDC * S], F32, tag="row")
                    denO = ps_row.tile([1, NDC * S], F32, tag="row")
                    nc.tensor.matmul(denE[:], onesb_c[0:S, :], pt[0:S, :], start=True, stop=True)
                    nc.tensor.matmul(denO[:], onesb_c[64:64 + S, :], pt[64:64 + S, :],
                                     start=True, stop=True, tile_position=(64, 0))
                    rdE = rden_p.tile([1, NDC * S], BF16, tag="rden")
                    rdO = rden_p.tile([1, NDC * S], BF16, tag="rden")
                    with nc.allow_low_precision(reason="softmax denom reciprocal"):
                        nc.vector.reciprocal(rdE[:], denE[:])
                        nc.vector.reciprocal(rdO[:], denO[:])
                    bc = ps_at.tile([128, NDC * S], F32, tag="at")
                    nc.tensor.matmul(bc[0:S, :], onesb_r[0:1, 0:S], rdE[:],
                                     start=True, stop=True, tile_position=(0, 0),
                                     skip_group_check=True)
                    nc.tensor.matmul(bc[64:64 + S, :], onesb_r[0:1, 0:S], rdO[:],
                                     start=True, stop=True, tile_position=(0, 64),
                                     skip_group_check=True)
                    pn = ptn_p.tile([128, NDC * S], BF16, tag="ptn")
                    nc.vector.tensor_tensor(out=pn[:], in0=pt[:], in1=bc[:], op=AOP.mult)
                    po = ps_at.tile([128, NDC * S], F32, tag="at")
                    for c in range(NDC):
                        nc.tensor.matmul(po[0:DK, c * S:(c + 1) * S],
                                         v[0:S, bi, (2 * c) * DK:(2 * c + 1) * DK],
                                         pn[0:S, c * S:(c + 1) * S],
                                         start=(c == 0), stop=(c == NDC - 1),
                                         tile_position=(0, 0), skip_group_check=True)
                    for c in range(NDC):
                        nc.tensor.matmul(po[64:128, c * S:(c + 1) * S],
                                         v[64:64 + S, bi, (2 * c + 1) * DK:(2 * c + 2) * DK],
                                         pn[64:64 + S, c * S:(c + 1) * S],
                                         start=(c == 0), stop=(c == NDC - 1),
                                         tile_position=(64, 64), skip_group_check=True)
                    nc.vector.tensor_copy(
                        ot[:, :, oc0:oc0 + S],
                        po[:].rearrange("p (c t) -> p c t", c=NDC))

                def wo_half(hf):
                    hc0 = hf * HT
                    ot = oTs[hf]
                    for m in range(NDC):
                        wt = wsm.tile([128, NDC, 128], BF16, tag="wsm")
                        nc.sync.dma_start(wt[:], d['wo_w'][li, m].rearrange("p (k c) -> p k c", k=NDC))
                        br = brow_p.tile([1, 128], BF16, tag="brow2")
                        nc.sync.dma_start(br[:], d['wo_b'][li, m])
                        ps = ps_mm.tile([128, HT], F32, tag="mm")
                        nc.tensor.matmul(ps[:], br[:], onesb_r[0:1, 0:HT], start=True, stop=False)
                        for k in range(NDC):
                            nc.tensor.matmul(ps[:], wt[:, k, :], ot[:, k, :],
                                             start=False, stop=(k == NDC - 1))
                        nc.vector.tensor_tensor(out=hT[:, m, hc0:hc0 + HT],
                                                in0=hT[:, m, hc0:hc0 + HT],
                                                in1=ps[:], op=AOP.add)

                for bi in range(HB):
                    attn_batch(bi)
                wo_half(0)
                for bi in range(HB, NB):
                    attn_batch(bi)
                wo_half(1)
            # ---- FFN ----
            if not last:
                aT2 = sb1.tile([128, NDC, T], BF16, tag="tagA")
                ln_full(hT, aT2)
                b1T = brow_p.tile([128, NFC], F32, tag="brow")
                nc.sync.dma_start(b1T[:], d['w1_bT'][li])
                if FP8_FFN:
                    aT8 = sb1.tile([128, NDC, T], FP8, tag="tagA8")
                    for c in range(NDC):
                        nc.scalar.activation(aT8[:, c, :], aT2[:, c, :], COPY,
                                             bias=0.0, scale=1.0)
                    fdt = FP8
                else:
                    aT8 = aT2
                    fdt = BF16
                ffq0 = sb1.tile([128, 8, T], fdt, tag="tagQ")
                ffq1 = sb1.tile([128, 8, T], fdt, tag="tagK")
                ffq2 = sb1.tile([128, 8, T], fdt, tag="tagVf")
                ffq3 = sb1.tile([128, 8, T], fdt, tag="tagO")
                ffq = [ffq0, ffq1, ffq2, ffq3]
                for m in range(NFC):
                    if FP8_FFN:
                        wt = wsm.tile([128, NDC // 2, 2, 128], FP8, tag="wsm8")
                        nc.sync.dma_start(wt[:], d['w1_w8'][li, m].rearrange(
                            "p (g j c) -> p g j c", g=NDC // 2, j=2))
                    else:
                        wt = wsm.tile([128, NDC, 128], BF16, tag="wsm")
                        nc.sync.dma_start(wt[:], d['w1_w'][li, m].rearrange("p (k c) -> p k c", k=NDC))
                    for hf in range(2):
                        ps = ps_mm.tile([128, HT], F32, tag="mm")
                        if FP8_FFN:
                            for g in range(NDC // 2):
                                nc.tensor.matmul(ps[:], wt[:, g, :, :],
                                                 aT8[:, 2 * g:2 * g + 2, hf * HT:(hf + 1) * HT],
                                                 start=(g == 0), stop=(g == NDC // 2 - 1),
                                                 perf_mode=DR)
                        else:
                            for k in range(NDC):
                                nc.tensor.matmul(ps[:], wt[:, k, :],
                                                 aT8[:, k, hf * HT:(hf + 1) * HT],
                                                 start=(k == 0), stop=(k == NDC - 1))
                        nc.scalar.activation(ffq[m // 8][:, m % 8, hf * HT:(hf + 1) * HT],
                                             ps[:], RELU, bias=b1T[:, m:m + 1],
                                             scale=(1.0 / 1024.0 if FP8_FFN else 1.0))
                for m in range(NDC):
                    if FP8_FFN:
                        w2t = wst.tile([128, NFC // 2, 2, 128], FP8, tag="wst")
                        nc.sync.dma_start(w2t[:], d['w2_w8'][li, m].rearrange(
                            "p (g j c) -> p g j c", g=NFC // 2, j=2))
                    else:
                        w2t = wst.tile([128, NFC, 128], BF16, tag="wst")
                        nc.sync.dma_start(w2t[:], d['w2_w'][li, m].rearrange("p (k c) -> p k c", k=NFC))
                    br = brow_p.tile([1, 128], BF16, tag="brow2")
                    nc.sync.dma_start(br[:], d['w2_b'][li, m])
                    for hf in range(2):
                        hc0 = hf * HT
                        ps = ps_mm.tile([128, HT], F32, tag="mm")
                        nc.tensor.matmul(ps[:], br[:], onesb_r[0:1, 0:HT], start=True, stop=False)
                        if FP8_FFN:
                            for g in range(NFC // 2):
                                nc.tensor.matmul(ps[:], w2t[:, g, :, :],
                                                 ffq[(2 * g) // 8][:, (2 * g) % 8:(2 * g) % 8 + 2,
                                                                   hc0:hc0 + HT],
                                                 start=False, stop=(g == NFC // 2 - 1),
                                                 perf_mode=DR)
                        else:
                            for k in range(NFC):
                                nc.tensor.matmul(ps[:], w2t[:, k, :],
                                                 ffq[k // 8][:, k % 8, hc0:hc0 + HT],
                                                 start=False, stop=(k == NFC - 1))
                        nc.vector.tensor_tensor(out=hT[:, m, hc0:hc0 + HT],
                                                in0=hT[:, m, hc0:hc0 + HT],
                                                in1=ps[:], op=AOP.add)
            else:
                # last layer: FFN only for the last token of each batch
                if not trim:
                    hL = sb1.tile([128, NDC, NB], F32R, tag="hL")
                    for c in range(NDC):
                        nc.vector.tensor_copy(
                            hL[:, c, :],
                            hT[:, c, :].rearrange("p (b s) -> p b s", s=S)[:, :, S - 1])
                aL = sb1.tile([128, NDC, NB], BF16, tag="aL")
                ln_half(hL, 0, NB, aL, NDC)
                b1T = brow_p.tile([128, NFC], F32, tag="brow")
                nc.sync.dma_start(b1T[:], d['w1_bT'][li])
                if FP8_FFN:
                    aL8 = sb1.tile([128, NDC, NB], FP8, tag="aL8")
                    nc.scalar.activation(aL8[:], aL[:], COPY, bias=0.0, scale=1.0)
                    fdt = FP8
                else:
                    aL8 = aL
                    fdt = BF16
                ffL = sb1.tile([128, NFC, NB], fdt, tag="ffL")
                for m in range(NFC):
                    if FP8_FFN:
                        wt = wsm.tile([128, NDC // 2, 2, 128], FP8, tag="wsm8")
                        nc.sync.dma_start(wt[:], d['w1_w8'][li, m].rearrange(
                            "p (g j c) -> p g j c", g=NDC // 2, j=2))
                    else:
                        wt = wsm.tile([128, NDC, 128], BF16, tag="wsm")
                        nc.sync.dma_start(wt[:], d['w1_w'][li, m].rearrange("p (k c) -> p k c", k=NDC))
                    ps = ps_mm.tile([128, NB], F32, tag="mm")
                    if FP8_FFN:
                        for g in range(NDC // 2):
                            nc.tensor.matmul(ps[:], wt[:, g, :, :],
                                             aL8[:, 2 * g:2 * g + 2, :],
                                             start=(g == 0), stop=(g == NDC // 2 - 1),
                                             perf_mode=DR)
                    else:
                        for k in range(NDC):
                            nc.tensor.matmul(ps[:], wt[:, k, :], aL8[:, k, :],
                                             start=(k == 0), stop=(k == NDC - 1))
                    nc.scalar.activation(ffL[:, m, :], ps[:], RELU,
                                         bias=b1T[:, m:m + 1],
                                         scale=(1.0 / 1024.0 if FP8_FFN else 1.0))
                for m in range(NDC):
                    if FP8_FFN:
                        w2t = wst.tile([128, NFC // 2, 2, 128], FP8, tag="wst")
                        nc.sync.dma_start(w2t[:], d['w2_w8'][li, m].rearrange(
                            "p (g j c) -> p g j c", g=NFC // 2, j=2))
                    else:
                        w2t = wst.tile([128, NFC, 128], BF16, tag="wst")
                        nc.sync.dma_start(w2t[:], d['w2_w'][li, m].rearrange("p (k c) -> p k c", k=NFC))
                    br = brow_p.tile([1, 128], BF16, tag="brow2")
                    nc.sync.dma_start(br[:], d['w2_b'][li, m])
                    ps = ps_mm.tile([128, NB], F32, tag="mm")
                    nc.tensor.matmul(ps[:], br[:], onesb_r[0:1, 0:NB], start=True, stop=False)
                    if FP8_FFN:
                        for g in range(NFC // 2):
                            nc.tensor.matmul(ps[:], w2t[:, g, :, :],
                                             ffL[:, 2 * g:2 * g + 2, :],
                                             start=False, stop=(g == NFC // 2 - 1),
                                             perf_mode=DR)
                    else:
                        for k in range(NFC):
                            nc.tensor.matmul(ps[:], w2t[:, k, :], ffL[:, k, :],
                                             start=False, stop=(k == NFC - 1))
                    nc.vector.tensor_tensor(out=hL[:, m, :], in0=hL[:, m, :],
                                            in1=ps[:], op=AOP.add)

        # ---------------- head ----------------
        if n_layers == L:
            src_pool = hL
        else:
            src_pool = sb1.tile([128, NDC, NB], F32R, tag="hL")
            for c in range(NDC):
                nc.vector.tensor_copy(
                    src_pool[:, c, :],
                    hT[:, c, :].rearrange("p (b s) -> p b s", s=S)[:, :, S - 1])
        pL = sb1.tile([128, NDC, NB], BF16, tag="pL")
        ln_half(src_pool, 0, NB, pL, NDC)
        cbT = brow_p.tile([128, NDC], F32, tag="brow")
        nc.sync.dma_start(cbT[:], d['cf_bT'][:])
        z1 = sb1.tile([128, NDC, NB], BF16, tag="z1")
        for m in range(NDC):
            wt = wsm.tile([128, NDC, 128], BF16, tag="wsm")
            nc.sync.dma_start(wt[:], d['cf_w'][m].rearrange("p (k c) -> p k c", k=NDC))
            ps = ps_mm.tile([128, NB], F32, tag="mm")
            for k in range(NDC):
                nc.tensor.matmul(ps[:], wt[:, k, :], pL[:, k, :],
                                 start=(k == 0), stop=(k == NDC - 1))
            nc.scalar.activation(z1[:, m, :], ps[:], RELU, bias=cbT[:, m:m + 1], scale=1.0)
        fwt = sb1.tile([128, NDC, NCLS], BF16, tag="fwt")
        nc.sync.dma_start(fwt[:], d['fc_w'].rearrange("p (k c) -> p k c", k=NDC))
        fb = brow_p.tile([NCLS, 1], F32, tag="brow2")
        nc.sync.dma_start(fb[:], d['fc_b'][:])
        ps = ps_mm.tile([NCLS, NB], F32, tag="mm")
        for k in range(NDC):
            nc.tensor.matmul(ps[:], fwt[:, k, :], z1[:, k, :],
                             start=(k == 0), stop=(k == NDC - 1))
        osb = sb1.tile([NCLS, NB], F32, tag="osb")
        nc.vector.tensor_scalar_add(osb[:], ps[:], fb[:])
        nc.sync.dma_start(out[:], osb[:])


def _prep_weights(inputs, n_layers=L):
    f64 = np.float64

    def prep_lhsT(W):
        # W [K, M] -> [M/128, 128, (K/128)*128] : tile[p, k*128+c] = W[k*128+p, mb*128+c]
        K, M = W.shape
        nk, nm = K // 128, M // 128
        return np.ascontiguousarray(
            W.reshape(nk, 128, nm, 128).transpose(2, 1, 0, 3).reshape(nm, 128, nk * 128)
        ).astype(BF16NP)

    emb = inputs['embed_w'].astype(f64)          # [1200, 1024]
    pos = np.arange(S, dtype=f64)[:, None]
    div = np.exp(np.arange(0, D, 2, dtype=np.float32).astype(f64) * (-math.log(10000.0) / D))
    pe = np.zeros((S, D), f64)
    pe[:, 0::2] = np.sin(pos * div)
    pe[:, 1::2] = np.cos(pos * div)
    Wp = np.zeros((NIP, D), f64)
    Wp[:NI] = emb
    Wp[NI:NI + S] = pe
    g = {}
    g['emb_w'] = prep_lhsT(Wp)

    ln_g = inputs['ln_g'].astype(f64); ln_b = inputs['ln_b'].astype(f64)
    aw = inputs['attn_w'].astype(f64); ab = inputs['attn_b'].astype(f64)
    fw1 = inputs['ff_w1'].astype(f64); fb1 = inputs['ff_b1'].astype(f64)
    fw2 = inputs['ff_w2'].astype(f64); fb2 = inputs['ff_b2'].astype(f64)

    qkv_w = np.zeros((L, 3, NDC, 128, NDC * 128), BF16NP)
    qkv_bT = np.zeros((L, 128, 3 * NDC), np.float32)
    wo_w = np.zeros((L, NDC, 128, NDC * 128), BF16NP)
    wo_b = np.zeros((L, NDC, 1, 128), BF16NP)
    wdt = FP8NP if FP8_FFN else BF16NP
    w1_w = np.zeros((L, NFC, 128, NDC * 128), wdt)
    w1_bT = np.zeros((L, 128, NFC), np.float32)
    w2_w = np.zeros((L, NDC, 128, NFC * 128), wdt)
    w2_b = np.zeros((L, NDC, 1, 128), BF16NP)

    def prep_dr(W, scale):
        # W [K, M] -> pair-interleaved DoubleRow layout
        # tile[p, g*256 + j*128 + c] = scale * W[(2g+j)*128 + p, m*128 + c]
        K, M = W.shape
        ng, nm = K // 256, M // 128
        arr = (W * scale).reshape(ng, 2, 128, nm, 128)
        return np.ascontiguousarray(
            arr.transpose(3, 2, 0, 1, 4).reshape(nm, 128, ng * 256)).astype(FP8NP)

    for i in range(n_layers):
        g1, b1 = ln_g[i, 0][:, None], ln_b[i, 0]
        for mat in range(3):
            We = g1 * aw[i, mat]
            be = ab[i, mat] + b1 @ aw[i, mat]
            qkv_w[i, mat] = prep_lhsT(We)
            if mat == 2:
                bv = be  # v bias folded into wo_b (softmax rows sum to 1)
            else:
                qkv_bT[i, :, mat * NDC:(mat + 1) * NDC] = be.reshape(NDC, 128).T
        wo_w[i] = prep_lhsT(aw[i, 3])
        wo_be = ab[i, 3] + bv @ aw[i, 3]
        wo_b[i] = wo_be.reshape(NDC, 1, 128).astype(BF16NP)
        g2, b2 = ln_g[i, 1][:, None], ln_b[i, 1]
        W1e = g2 * fw1[i]
        b1e = fb1[i] + b2 @ fw1[i]
        if FP8_FFN:
            # FFN1 weights x64 for fp8 range; relu scale 1/1024 makes
            # ffq = relu_out / 16; FFN2 weights x16 restores the product.
            w1_w[i] = prep_dr(W1e, 64.0)
            w1_bT[i] = (b1e / 16.0).reshape(NFC, 128).T
            w2_w[i] = prep_dr(fw2[i], 16.0)
        else:
            w1_w[i] = prep_lhsT(W1e)
            w1_bT[i] = b1e.reshape(NFC, 128).T
            w2_w[i] = prep_lhsT(fw2[i])
        w2_b[i] = fb2[i].reshape(NDC, 1, 128).astype(BF16NP)

    g['qkv_w'] = qkv_w; g['qkv_bT'] = qkv_bT
    g['wo_w'] = wo_w; g['wo_b'] = wo_b
    g['w1_bT'] = w1_bT; g['w2_b'] = w2_b
    if FP8_FFN:
        g['w1_w8'] = w1_w; g['w2_w8'] = w2_w
    else:
        g['w1_w'] = w1_w; g['w2_w'] = w2_w

    inv = 1.0 / math.sqrt(1.0 + 1e-5)
    fin_g = inputs['fin_g'].astype(f64); fin_b = inputs['fin_b'].astype(f64)
    A1 = fin_g * inv * inputs['cf_bn_g'].astype(f64)
    C1 = fin_b * inv * inputs['cf_bn_g'].astype(f64) + inputs['cf_bn_b'].astype(f64)
    cfw = inputs['cf_w'].astype(f64)
    cf_we = A1[:, None] * cfw
    cf_be = inputs['cf_b'].astype(f64) + C1 @ cfw
    g['cf_w'] = prep_lhsT(cf_we)
    g['cf_bT'] = cf_be.reshape(NDC, 128).T.astype(np.float32)
    A2 = inv * inputs['fc_bn_g'].astype(f64)
    C2 = inputs['fc_bn_b'].astype(f64)
    fcw = inputs['fc_w'].astype(f64)
    fc_we = A2[:, None] * fcw
    fc_be = inputs['fc_b'].astype(f64) + C2 @ fcw
    g['fc_w'] = np.ascontiguousarray(
        fc_we.reshape(NDC, 128, NCLS).transpose(1, 0, 2).reshape(128, NDC * NCLS)
    ).astype(BF16NP)
    g['fc_b'] = fc_be.reshape(NCLS, 1).astype(np.float32)
    g['ones'] = np.ones((128, 512), np.float32)
    g['onesb'] = np.ones((128, 512), BF16NP)
    g['ident'] = np.eye(128, dtype=BF16NP)
    return g


def _run_timed(nc, in_maps, n_iters=300):
    """Mirror bass2jax.run_bass_via_pjrt (no donation), time steady-state execs."""
    import time
    import jax
    import numpy as _np
    from jax.experimental.shard_map import shard_map
    from jax.sharding import Mesh, PartitionSpec, NamedSharding
    from concourse import bass2jax as b2j
    from concourse import mybir as _mb

    b2j.install_neuronx_cc_hook()
    n_cores = len(in_maps)
    partition_name = nc.partition_id_tensor.name if nc.partition_id_tensor else None
    in_names, out_names, out_avals, zero_outs = [], [], [], []
    for alloc in nc.m.functions[0].allocations:
        if not isinstance(alloc, _mb.MemoryLocationSet):
            continue
        name = alloc.memorylocations[0].name
        if alloc.kind == "ExternalInput":
            if name != partition_name:
                in_names.append(name)
        elif alloc.kind == "ExternalOutput":
            shape = tuple(alloc.tensor_shape)
            dtype = _mb.dt.np(alloc.dtype)
            out_names.append(name)
            out_avals.append(jax.core.ShapedArray(shape, dtype))
            zero_outs.append(_np.zeros(shape, dtype))
    n_params = len(in_names)
    all_in_names = list(in_names) + list(out_names)
    if partition_name is not None:
        all_in_names.append(partition_name)

    def _body(*args):
        operands = list(args)
        if partition_name is not None:
            operands.append(b2j.partition_id_tensor())
        outs = b2j._bass_exec_p.bind(
            *operands,
            out_avals=tuple(out_avals),
            in_names=tuple(all_in_names),
            out_names=tuple(out_names),
            lowering_input_output_aliases=(),
            sim_require_finite=True,
            sim_require_nnan=True,
            nc=nc,
        )
        return tuple(outs)

    devices = jax.devices()[:n_cores]
    mesh = Mesh(_np.asarray(devices), ("core",))
    spec = PartitionSpec("core")
    sharded = jax.jit(shard_map(
        _body, mesh=mesh, in_specs=(spec,) * (n_params + len(out_names)),
        out_specs=(spec,) * len(out_names), check_rep=False))
    sh = NamedSharding(mesh, spec)
    concat_in = [
        jax.device_put(_np.concatenate([_np.asarray(m[name]) for m in in_maps], axis=0), sh)
        for name in in_names
    ]
    concat_zeros = [
        jax.device_put(_np.zeros((n_cores * z.shape[0], *z.shape[1:]), z.dtype), sh)
        for z in zero_outs
    ]
    outs = sharded(*concat_in, *concat_zeros)
    jax.block_until_ready(outs)
    t0 = time.time()
    for _ in range(n_iters):
        outs = sharded(*concat_in, *concat_zeros)
    jax.block_until_ready(outs)
    t1 = time.time()
    per_call_ns = (t1 - t0) / n_iters * 1e9
    results = [
        {name: _np.asarray(outs[i]).reshape(n_cores, *out_avals[i].shape)[c]
         for i, name in enumerate(out_names)}
        for c in range(n_cores)
    ]
    return results, per_call_ns


def _make_in_maps(inputs, g):
    x = inputs['x']
    xr = np.asarray(x).reshape(B, S, NI)
    in_maps = []
    for ci in range(NCORES):
        xc = xr[ci * NB:(ci + 1) * NB].astype(np.float64)  # [16, 50, 1200]
        xa = np.zeros((NB, S, NIP), np.float32)
        xa[:, :, :NI] = xc
        xa[np.arange(NB)[:, None], np.arange(S)[None, :], NI + np.arange(S)[None, :]] = 1.0
        # xT [NKI, 128, T]: feature-major, tokens (b, s)
        xT = np.ascontiguousarray(
            xa.reshape(T, NIP).T.reshape(NKI, 128, T)).astype(BF16NP)
        m = dict(g)
        m['xT'] = xT
        in_maps.append(m)
    return in_maps


def kernel(**inputs):
    global LAST_EXEC_NS
    n_layers = int(inputs.pop('_n_layers', L))
    if n_layers not in _CACHE:
        _CACHE[n_layers] = _build(n_layers)
    nc = _CACHE[n_layers]
    g = _prep_weights(inputs, n_layers)
    in_maps = _make_in_maps(inputs, g)

    results, per_call_ns = _run_timed(nc, in_maps)
    LAST_EXEC_NS = int(per_call_ns)
    outs = [r['out'].T for r in results]   # each [NB, NCLS]
    return np.concatenate(outs, axis=0).astype(np.float32)

